# revision 8
# baseline (speedup 1.0000x reference)
"""Trainium2 Bass kernel for AttentionPoolCompressor.

Computation (matches the reference nn.Module):
    x = chunk.reshape(N, 4, 512)
    scores = einsum('d,nrd->nr', query, x) / sqrt(512)
    attn   = softmax(scores, axis=-1)
    pooled = einsum('nr,nrd->nd', attn, x)
    out    = pooled @ w.T + b

Sharding: chunk rows are split contiguously across 8 NeuronCores (each pools
its own L/8 rows independently); query / w / b are replicated.  No
collectives are needed; each core writes its own slice of the output.

Per-core pipeline (64 tiles of 512 input rows -> 128 output rows each):
  1. DMA one tile X[128, 2048] (partition n holds its 4 input rows,
     8KB contiguous per partition).
  2. Scores: GpSimdE computes P = X * q_tiled4 (elementwise; GpSimd is slower
     than VectorE at this but otherwise idle, and the custom fused DVE op
     that would do mul+reduce in one pass crashes this runtime), then
     VectorE reduces P viewed as [128, 4, 512] over the innermost axis ->
     S[128, 4] with n on partitions.  The 1/sqrt(d) scale is folded into the
     Exp activation's scale.
  3. Softmax: exp on ScalarE (per-partition scale + free accum_out row-sum);
     reciprocal on VectorE.  The max-subtraction is skipped deliberately:
     scores are q.x/sqrt(d) with q ~ 0.02*N(0,1), so |s| < 0.15 and exp
     cannot overflow; the softmax ratios are identical either way.
  4. Pooling on TensorE: build D_r = diag(E_r * rec) via 2-scalar
     tensor_scalar against an identity kept in PSUM (so VectorE reads the
     PSUM port, never the SBUF port it would otherwise share with the
     streaming GpSimdE), then 4 accumulating matmuls pooled += D_r.T @ X_r
     (float32r, N=512 -> 1 cycle/row).
  5. pooled (PSUM) -> SBUF on ScalarE, 4x PE transposes -> pooledT,
     -> SBUF on ScalarE.
  6. Projection on TensorE: 4 accumulating matmuls pooledT_c.T @ wT_c
     (wT pre-transposed on host), plus a K=1 ones x b matmul that adds the
     bias inside the same PSUM accumulation group.
  7. ScalarE copies PSUM -> SBUF, DMA out.
"""

import math
import sys

import numpy as np

if "/opt/trn_rl_repo" not in sys.path:
    sys.path.insert(0, "/opt/trn_rl_repo")

D = 512
RATIO = 4
N_CORES = 8
L_FULL = 262144
ROWS_PER_CORE = L_FULL // N_CORES  # 32768
TILE_ROWS = 512  # input rows per tile -> 128 output rows
OUT_ROWS_PER_CORE = ROWS_PER_CORE // RATIO  # 8192

_NC_CACHE = {}


def _build_nc(rows_per_core):
    from contextlib import ExitStack

    import concourse.bacc as bacc
    import concourse.tile as tile
    from concourse import mybir

    fp32 = mybir.dt.float32
    f32r = mybir.dt.float32r
    Alu = mybir.AluOpType
    Act = mybir.ActivationFunctionType

    n_tiles = rows_per_core // TILE_ROWS
    out_rows = rows_per_core // RATIO
    inv_sqrt_d = 1.0 / math.sqrt(D)

    nc = bacc.Bacc("TRN2", target_bir_lowering=False, debug=False)
    # Tensors that feed float32r matmuls are declared float32r end-to-end
    # (same byte layout as fp32); walrus' checkMatmultFP32r requires matmul
    # operands to be produced as fp32r.
    chunk = nc.dram_tensor("chunk", [rows_per_core, D], f32r, kind="ExternalInput").ap()
    wT = nc.dram_tensor("wT", [D, D], f32r, kind="ExternalInput").ap()
    qbc4 = nc.dram_tensor("qbc4", [128, RATIO * D], fp32, kind="ExternalInput").ap()
    ident = nc.dram_tensor("ident", [128, 128], fp32, kind="ExternalInput").ap()
    ones1 = nc.dram_tensor("ones1", [1, 128], f32r, kind="ExternalInput").ap()
    brow = nc.dram_tensor("brow", [1, D], f32r, kind="ExternalInput").ap()
    out = nc.dram_tensor("out", [out_rows, D], fp32, kind="ExternalOutput").ap()

    with tile.TileContext(nc) as tc, ExitStack() as ctx:
        const = ctx.enter_context(tc.tile_pool(name="const", bufs=1))
        xp = ctx.enter_context(tc.tile_pool(name="xp", bufs=4))
        prodp = ctx.enter_context(tc.tile_pool(name="prodp", bufs=3))
        sp = ctx.enter_context(tc.tile_pool(name="sp", bufs=3))
        dp = ctx.enter_context(tc.tile_pool(name="dp", bufs=3))
        pooledp = ctx.enter_context(tc.tile_pool(name="pooledp", bufs=2))
        ptp = ctx.enter_context(tc.tile_pool(name="ptp", bufs=2))
        outp = ctx.enter_context(tc.tile_pool(name="outp", bufs=3))
        psc = ctx.enter_context(tc.tile_pool(name="psc", bufs=1, space="PSUM"))
        ps1 = ctx.enter_context(tc.tile_pool(name="ps1", bufs=2, space="PSUM"))
        ps2 = ctx.enter_context(tc.tile_pool(name="ps2", bufs=2, space="PSUM"))
        ps3 = ctx.enter_context(tc.tile_pool(name="ps3", bufs=2, space="PSUM"))

        qbc4_t = const.tile([128, RATIO * D], fp32)
        nc.sync.dma_start(out=qbc4_t[:], in_=qbc4[:, :])
        wt_t = const.tile([128, 4 * D], f32r)
        for k in range(4):
            nc.sync.dma_start(
                out=wt_t[:, k * D : (k + 1) * D], in_=wT[k * 128 : (k + 1) * 128, :]
            )
        id_t = const.tile([128, 128], fp32)
        nc.sync.dma_start(out=id_t[:], in_=ident[:, :])
        ones_t = const.tile([1, 128], f32r)
        nc.sync.dma_start(out=ones_t[:], in_=ones1[:, :])
        b_t = const.tile([1, D], f32r)
        nc.sync.dma_start(out=b_t[:], in_=brow[:, :])
        # Identity parked in PSUM: the diag-build tensor_scalar then reads
        # the PSUM port instead of the SBUF port shared with GpSimdE.
        i_ps = psc.tile([128, 128], fp32)
        nc.tensor.transpose(i_ps[:], id_t[:], id_t[:])

        for t in range(n_tiles):
            x_t = xp.tile([128, RATIO * D], f32r)
            nc.sync.dma_start(
                out=x_t[:],
                in_=chunk[t * TILE_ROWS : (t + 1) * TILE_ROWS, :].rearrange(
                    "(p r) d -> p (r d)", r=RATIO
                ),
            )

            p_t = prodp.tile([128, RATIO * D], fp32)
            nc.gpsimd.tensor_tensor(
                p_t[:], x_t[:].bitcast(fp32), qbc4_t[:], Alu.mult
            )

            s_t = sp.tile([128, RATIO], fp32)
            nc.vector.tensor_reduce(
                s_t[:],
                p_t[:].rearrange("p (r d) -> p r d", r=RATIO),
                axis=mybir.AxisListType.X,
                op=Alu.add,
            )

            e_t = sp.tile([128, RATIO], fp32)
            sum_t = sp.tile([128, 1], fp32)
            nc.scalar.activation(
                out=e_t[:], in_=s_t[:], func=Act.Exp, scale=inv_sqrt_d,
                accum_out=sum_t[:],
            )
            rec_t = sp.tile([128, 1], fp32)
            nc.vector.reciprocal(rec_t[:], sum_t[:])

            d_t = dp.tile([128, RATIO * 128], f32r)
            for r in range(RATIO):
                nc.vector.tensor_scalar(
                    out=d_t[:, r * 128 : (r + 1) * 128],
                    in0=i_ps[:],
                    scalar1=e_t[:, r : r + 1],
                    scalar2=rec_t[:],
                    op0=Alu.mult,
                    op1=Alu.mult,
                )

            pooled_ps = ps1.tile([128, D], fp32)
            for r in range(RATIO):
                nc.tensor.matmul(
                    out=pooled_ps[:],
                    lhsT=d_t[:, r * 128 : (r + 1) * 128],
                    rhs=x_t[:, r * D : (r + 1) * D],
                    start=(r == 0),
                    stop=(r == RATIO - 1),
                )
            pooled_sb = pooledp.tile([128, D], fp32)
            nc.scalar.copy(pooled_sb[:], pooled_ps[:])

            pt_ps = ps2.tile([128, D], fp32)
            for c in range(4):
                nc.tensor.transpose(
                    pt_ps[:, c * 128 : (c + 1) * 128],
                    pooled_sb[:, c * 128 : (c + 1) * 128],
                    id_t[:],
                )
            pt_sb = ptp.tile([128, D], f32r)
            nc.scalar.copy(pt_sb[:], pt_ps[:])

            o_ps = ps3.tile([128, D], fp32)
            for c in range(4):
                nc.tensor.matmul(
                    out=o_ps[:],
                    lhsT=pt_sb[:, c * 128 : (c + 1) * 128],
                    rhs=wt_t[:, c * D : (c + 1) * D],
                    start=(c == 0),
                    stop=False,
                )
            nc.tensor.matmul(
                out=o_ps[:],
                lhsT=ones_t[:],
                rhs=b_t[:],
                start=False,
                stop=True,
            )
            out_sb = outp.tile([128, D], fp32)
            nc.scalar.copy(out_sb[:], o_ps[:])
            nc.sync.dma_start(
                out=out[t * 128 : (t + 1) * 128, :], in_=out_sb[:]
            )

    # Bacc.compile runs the lowering passes raw Bass lacks: sync-wait
    # splitting (HW allows at most 1 wait/instruction) and InstISA byte
    # encoding ("ISA wrong length" otherwise).
    nc.compile()
    return nc


def get_nc(rows_per_core=ROWS_PER_CORE):
    key = rows_per_core
    if key not in _NC_CACHE:
        _NC_CACHE[key] = _build_nc(rows_per_core)
    return _NC_CACHE[key]


def _aux_inputs(query, w, b):
    q = np.asarray(query, dtype=np.float32)
    qbc4 = np.ascontiguousarray(
        np.broadcast_to(np.tile(q, RATIO), (128, RATIO * D))
    )
    wT = np.ascontiguousarray(np.asarray(w, dtype=np.float32).T)
    ident = np.eye(128, dtype=np.float32)
    ones1 = np.ones((1, 128), dtype=np.float32)
    brow = np.asarray(b, dtype=np.float32).reshape(1, D)
    return {"qbc4": qbc4, "wT": wT, "ident": ident, "ones1": ones1, "brow": brow}


def make_in_maps(chunk, query, w, b, rows_per_core=ROWS_PER_CORE, n_cores=N_CORES):
    chunk = np.asarray(chunk, dtype=np.float32)
    aux = _aux_inputs(query, w, b)
    return [
        {"chunk": np.ascontiguousarray(chunk[c * rows_per_core : (c + 1) * rows_per_core]), **aux}
        for c in range(n_cores)
    ]


def kernel(chunk, query, w, b, trace=False):
    from concourse.bass_utils import run_bass_kernel_spmd

    nc = get_nc(ROWS_PER_CORE)
    in_maps = make_in_maps(chunk, query, w, b)
    res = run_bass_kernel_spmd(nc, in_maps, list(range(N_CORES)), trace=trace)
    out = np.concatenate([res.results[c]["out"] for c in range(N_CORES)], axis=0)
    kernel.last_results = res
    return out
